# revision 11
# baseline (speedup 1.0000x reference)
"""Causal self-attention (B=2, T=2048, C=1024, H=16) on 8 TRN2 NeuronCores.

Tensor-parallel over heads: core i computes heads 2i, 2i+1 end-to-end
(qkv projection slice, rotary+rms, attention, c_proj row-slice) and returns a
partial [4096, 1024] output; the host sums the 8 partials and adds b_proj.

Layouts (per core):
  qN/kN/vT: [128, ntok] sbuf, rows = (h_local*64 + d), cols = b*T + t.
  Rotary half-swap is a PE permutation matmul (cross-partition moves are
  impossible on DVE); rms mean-square uses a selector-ones matmul; the
  rsqrt is exp(-0.5*ln(ms+eps)) on ACT (Rsqrt table is banned).
  Scores are computed transposed, S^T[kv, q], in f32r (tf32-like, full PE
  rate); softmax runs without max-subtraction (rms-normed q,k bound
  |score| <= sqrt(D)); causal masking is applied post-exp via gpsimd
  affine_select; the denominator comes from a ones-column appended to V;
  1/denom is applied per-head when combining c_proj psum results.
"""
import sys

sys.path.insert(0, '/opt/trn_rl_repo')

import numpy as np
from contextlib import ExitStack

import concourse.bass as bass
import concourse.tile as tile
from concourse import bacc, mybir
from concourse.bass_utils import run_bass_kernel_spmd

F32 = mybir.dt.float32
F32R = mybir.dt.float32r

N_HEAD = 16
N_CORES = 8
B, T, C = 2, 2048, 1024
D = C // N_HEAD          # 64
NTOK = B * T             # 4096
EPS = float(np.finfo(np.float32).eps)

QB = 512                 # query block (free dim of S^T tiles)
KVB = 128                # kv block (partition dim of S^T tiles)
STG = 2                  # kv-blocks per exp group (psum banks per S^T tile)


def build_nc(ntok=NTOK):
    """Build the SPMD Bass program. ntok can be reduced for simulation."""
    nb = ntok // T if ntok >= T else 1          # batches
    tseq = ntok // nb                           # tokens per batch
    n_tb = ntok // QB                           # 512-token blocks (proj)
    n_tt = ntok // 128                          # 128-token tiles (c_proj)
    n_qb = tseq // QB                           # query blocks per batch
    n_kv = tseq // KVB                          # kv blocks per batch

    nc = bacc.Bacc("TRN2", target_bir_lowering=False, debug=False)

    # ---- DRAM I/O ----
    xT_d = nc.dram_tensor("xT", [C, ntok], F32R, kind="ExternalInput").ap()
    wq_d = nc.dram_tensor("wq", [C, 128], F32R, kind="ExternalInput").ap()
    wk_d = nc.dram_tensor("wk", [C, 128], F32R, kind="ExternalInput").ap()
    wv_d = nc.dram_tensor("wv", [C, 128], F32R, kind="ExternalInput").ap()
    bq_d = nc.dram_tensor("bq", [128, 1], F32, kind="ExternalInput").ap()
    bk_d = nc.dram_tensor("bk", [128, 1], F32, kind="ExternalInput").ap()
    bv_d = nc.dram_tensor("bv", [128, 1], F32, kind="ExternalInput").ap()
    wp_d = nc.dram_tensor("wp", [128, C], F32R, kind="ExternalInput").ap()
    cosb_d = nc.dram_tensor("cosb", [128, ntok], F32, kind="ExternalInput").ap()
    sinb_d = nc.dram_tensor("sinb", [128, ntok], F32, kind="ExternalInput").ap()
    p2t_d = nc.dram_tensor("p2t", [128, 128], F32R, kind="ExternalInput").ap()
    selms_d = nc.dram_tensor("selms", [128, 2], F32R, kind="ExternalInput").ap()
    selbc_d = nc.dram_tensor("selbc", [2, 128], F32R, kind="ExternalInput").ap()
    ident_d = nc.dram_tensor("ident", [128, 64], F32R, kind="ExternalInput").ap()
    ones1_d = nc.dram_tensor("ones1", [128, 1], F32R, kind="ExternalInput").ap()
    out_d = nc.dram_tensor("out", [ntok, C], F32, kind="ExternalOutput").ap()
    dscr_d = nc.dram_tensor("dscr", [2, ntok], F32, kind="Internal").ap()

    with tile.TileContext(nc) as tc, ExitStack() as ctx:
        consts = ctx.enter_context(tc.tile_pool(name="consts", bufs=1))
        big = ctx.enter_context(tc.tile_pool(name="big", bufs=1))

        # ---- constants / weights ----
        wq = consts.tile([128, 8, 128], F32R, tag="wq")
        wk = consts.tile([128, 8, 128], F32R, tag="wk")
        wv = consts.tile([128, 8, 128], F32R, tag="wv")
        for w_s, w_d in ((wq, wq_d), (wk, wk_d), (wv, wv_d)):
            # sbuf[p, ci, f] = W[ci*128+p, f]
            nc.sync.dma_start(w_s[:], w_d.rearrange("(ci p) f -> p ci f", p=128))
        bq = consts.tile([128, 1], F32, tag="bq")
        bk = consts.tile([128, 1], F32, tag="bk")
        bv = consts.tile([128, 1], F32, tag="bv")
        nc.sync.dma_start(bq[:], bq_d)
        nc.sync.dma_start(bk[:], bk_d)
        nc.sync.dma_start(bv[:], bv_d)
        wp = consts.tile([128, C], F32R, tag="wp")
        nc.sync.dma_start(wp[:], wp_d)
        cosb = consts.tile([128, ntok], F32, tag="cosb")
        sinb = consts.tile([128, ntok], F32, tag="sinb")
        nc.sync.dma_start(cosb[:], cosb_d)
        nc.sync.dma_start(sinb[:], sinb_d)
        p2t = consts.tile([128, 128], F32R, tag="p2t")
        nc.sync.dma_start(p2t[:], p2t_d)
        selms = consts.tile([128, 2], F32R, tag="selms")
        nc.sync.dma_start(selms[:], selms_d)
        selbc = consts.tile([2, 128], F32R, tag="selbc")
        nc.sync.dma_start(selbc[:], selbc_d)
        ident = consts.tile([128, 64], F32R, tag="ident")
        nc.sync.dma_start(ident[:], ident_d)
        ones1 = consts.tile([128, 1], F32R, tag="ones1")
        nc.sync.dma_start(ones1[:], ones1_d)
        epsb = consts.tile([2, 1], F32, tag="epsb")
        nc.vector.memset(epsb[:], EPS)

        # ---- persistent big tensors ----
        q_raw = big.tile([128, ntok], F32R, tag="q_raw")
        k_raw = big.tile([128, ntok], F32R, tag="k_raw")
        qN = q_raw       # normalized+rotated result overwrites raw in place
        kN = k_raw
        vT = big.tile([128, ntok], F32R, tag="vT")
        yTn = big.tile([128, ntok], F32R, tag="yTn")
        denomT = big.tile([128, n_tt], F32, tag="denomT")
        denomT2 = big.tile([128, n_tt], F32, tag="denomT2")
        recipT = big.tile([128, n_tt], F32, tag="recipT")
        recipT2 = big.tile([128, n_tt], F32, tag="recipT2")

        # ================= Phase 1: qkv projection =================
        with tc.tile_pool(name="xtp", bufs=24) as xtp, \
             tc.tile_pool(name="prj_ps", bufs=3, space="PSUM") as prj_ps:
            for tb in range(n_tb):
                xts = []
                for ci in range(8):
                    xt = xtp.tile([128, QB], F32R, tag="xt")
                    nc.sync.dma_start(
                        xt[:], xT_d[ci * 128:(ci + 1) * 128, tb * QB:(tb + 1) * QB])
                    xts.append(xt)
                for w_s, b_s, dst in ((wq, bq, q_raw), (wk, bk, k_raw),
                                      (wv, bv, vT)):
                    ps = prj_ps.tile([128, QB], F32, tag="prj")
                    for ci in range(8):
                        nc.tensor.matmul(ps[:], w_s[:, ci, :], xts[ci][:],
                                         start=(ci == 0), stop=(ci == 7))
                    nc.vector.tensor_scalar_add(
                        dst[:, tb * QB:(tb + 1) * QB], ps[:], b_s[:])

        # ================= Phase 2: rms stats + rotary (per chunk) ==========
        CH = min(1024, ntok)
        n_ch = ntok // CH
        with tc.tile_pool(name="rr_ps", bufs=2, space="PSUM") as rr_ps, \
             tc.tile_pool(name="ms_ps", bufs=2, space="PSUM") as ms_ps, \
             tc.tile_pool(name="rr_sb", bufs=2) as rr_sb:
            for raw, dst in ((q_raw, qN), (k_raw, kN)):
                for ch in range(n_ch):
                    sl = slice(ch * CH, (ch + 1) * CH)
                    sqc = rr_sb.tile([128, CH], F32R, tag="sqc")
                    nc.vector.tensor_mul(sqc[:], raw[:, sl], raw[:, sl])
                    lmsc = rr_sb.tile([2, CH], F32, tag="lmsc")
                    for half in range(CH // QB):
                        hs = slice(half * QB, (half + 1) * QB)
                        msp = ms_ps.tile([2, QB], F32, tag="ms")
                        nc.tensor.matmul(msp[:], selms[:], sqc[:, hs],
                                         start=True, stop=True)
                        nc.scalar.activation(lmsc[:, hs], msp[:],
                                             mybir.ActivationFunctionType.Ln,
                                             bias=epsb[:])
                    rqkc = rr_sb.tile([2, CH], F32R, tag="rqkc")
                    nc.scalar.activation(rqkc[:], lmsc[:],
                                         mybir.ActivationFunctionType.Exp,
                                         scale=-0.5)
                    swp = rr_ps.tile([128, CH], F32, tag="swp")
                    for half in range(CH // QB):
                        hs = slice(half * QB, (half + 1) * QB)
                        s2 = slice(ch * CH + half * QB, ch * CH + (half + 1) * QB)
                        nc.tensor.matmul(swp[:, hs], p2t[:], raw[:, s2],
                                         start=True, stop=True)
                    ts_ = rr_sb.tile([128, CH], F32, tag="ts")
                    nc.vector.tensor_mul(ts_[:], swp[:], sinb[:, sl])
                    rot = rr_sb.tile([128, CH], F32, tag="rot")
                    nc.vector.tensor_mul(rot[:], raw[:, sl], cosb[:, sl])
                    nc.vector.tensor_add(rot[:], rot[:], ts_[:])
                    bcp = rr_ps.tile([128, CH], F32, tag="swp")
                    for half in range(CH // QB):
                        hs = slice(half * QB, (half + 1) * QB)
                        nc.tensor.matmul(bcp[:, hs], selbc[:], rqkc[:, hs],
                                         start=True, stop=True)
                    nc.vector.tensor_mul(dst[:, sl], rot[:], bcp[:])

        # ================= Phase 3: attention + c_proj =================
        scale = 1.0 / float(np.sqrt(D))
        with tc.tile_pool(name="st_ps", bufs=2, space="PSUM") as st_ps, \
             tc.tile_pool(name="acc_ps", bufs=3, space="PSUM") as acc_ps, \
             tc.tile_pool(name="vt_ps", bufs=1, space="PSUM") as vt_ps, \
             tc.tile_pool(name="pt_sb", bufs=3) as pt_sb, \
             tc.tile_pool(name="vv_sb", bufs=n_kv) as vv_sb, \
             tc.tile_pool(name="out_sb", bufs=3) as out_sb, \
             tc.tile_pool(name="dn_sb", bufs=2) as dn_sb:
            for b in range(nb):
                for h in range(2):
                    hsl = slice(h * 64, (h + 1) * 64)
                    tb0 = b * tseq
                    # -- V transpose to token-major + ones column --
                    vts = []
                    for kv in range(n_kv):
                        vp = vt_ps.tile([128, 64], F32R, tag="vtr")
                        nc.tensor.transpose(
                            vp[:], vT[hsl, tb0 + kv * KVB:tb0 + (kv + 1) * KVB],
                            ident[hsl, :])
                        vv = vv_sb.tile([128, 65], F32R, tag="vv")
                        nc.vector.tensor_copy(vv[:, 0:64], vp[:])
                        nc.vector.tensor_copy(vv[:, 64:65], ones1[:])
                        vts.append(vv)
                    # -- per query block --
                    for qb in range(n_qb):
                        q0 = tb0 + qb * QB
                        nkvb = (qb + 1) * (QB // KVB)
                        ya = acc_ps.tile([65, QB], F32, tag="acc")
                        for g in range((nkvb + STG - 1) // STG):
                            gw = min(STG, nkvb - g * STG)
                            st = st_ps.tile([128, STG * QB], F32, tag="st")
                            for j in range(gw):
                                kv = g * STG + j
                                nc.tensor.matmul(
                                    st[:, j * QB:(j + 1) * QB],
                                    kN[hsl, tb0 + kv * KVB:tb0 + (kv + 1) * KVB],
                                    qN[hsl, q0:q0 + QB],
                                    start=True, stop=True)
                            pt = pt_sb.tile([128, STG * QB], F32R, tag="pt")
                            nc.scalar.activation(
                                pt[:, 0:gw * QB], st[:, 0:gw * QB],
                                mybir.ActivationFunctionType.Exp, scale=scale)
                            for j in range(gw):
                                kv = g * STG + j
                                dq = qb * QB - kv * KVB
                                if dq < QB:
                                    nc.gpsimd.affine_select(
                                        out=pt[:, j * QB:(j + 1) * QB],
                                        in_=pt[:, j * QB:(j + 1) * QB],
                                        compare_op=mybir.AluOpType.is_ge,
                                        fill=0.0, base=dq,
                                        pattern=[[1, QB]], channel_multiplier=-1)
                                nc.tensor.matmul(
                                    ya[:], vts[kv][:], pt[:, j * QB:(j + 1) * QB],
                                    start=(kv == 0), stop=(kv == nkvb - 1))
                        nc.vector.tensor_copy(yTn[hsl, q0:q0 + QB], ya[0:64, :])
                        dnr = dn_sb.tile([1, QB], F32, tag="dnr")
                        nc.vector.tensor_copy(dnr[:], ya[64:65, :])
                        nc.sync.dma_start(dscr_d[h:h + 1, q0:q0 + QB], dnr[:])
                    dt_ = denomT if h == 0 else denomT2
                    rt_ = recipT if h == 0 else recipT2
                    c0 = tb0 // 128
                    nc.sync.dma_start(
                        dt_[:, c0:c0 + tseq // 128],
                        dscr_d[h, tb0:tb0 + tseq].rearrange("(j p) -> p j", p=128))
                    nc.vector.reciprocal(rt_[:, c0:c0 + tseq // 128],
                                         dt_[:, c0:c0 + tseq // 128])

                # -- c_proj for this batch (both heads ready) --
                for tt in range(tseq // 128):
                    t0 = b * tseq + tt * 128
                    ct = t0 // 128
                    ost = out_sb.tile([128, C], F32, tag="ost")
                    for oc in range(2):
                        cpA = acc_ps.tile([128, 512], F32, tag="acc")
                        nc.tensor.matmul(
                            cpA[:], yTn[0:64, t0:t0 + 128],
                            wp[0:64, oc * 512:(oc + 1) * 512],
                            start=True, stop=True)
                        cpB = acc_ps.tile([128, 512], F32, tag="acc")
                        nc.tensor.matmul(
                            cpB[:], yTn[64:128, t0:t0 + 128],
                            wp[64:128, oc * 512:(oc + 1) * 512],
                            start=True, stop=True)
                        osl = ost[:, oc * 512:(oc + 1) * 512]
                        nc.vector.tensor_scalar_mul(
                            osl, cpA[:], recipT[:, ct:ct + 1])
                        nc.vector.scalar_tensor_tensor(
                            out=osl, in0=cpB[:], scalar=recipT2[:, ct:ct + 1],
                            in1=osl, op0=mybir.AluOpType.mult,
                            op1=mybir.AluOpType.add)
                    nc.sync.dma_start(out_d[t0:t0 + 128, :], ost[:])

    nc.compile()
    return nc


def host_inputs(x, cos, sin, W_attn, b_attn, W_proj, ntok=NTOK):
    """Build the 8 per-core input maps from full inputs."""
    nb = ntok // T if ntok >= T else 1
    tseq = ntok // nb
    xT = np.ascontiguousarray(x.reshape(ntok, C).T).astype(np.float32)
    cosT = np.tile(cos[:tseq].T, (1, nb)).astype(np.float32)   # [32, ntok]
    sinT = np.tile(sin[:tseq].T, (1, nb)).astype(np.float32)
    cosb = np.tile(cosT, (4, 1)).astype(np.float32)            # [128, ntok]
    sinb = np.tile(sinT, (4, 1)).astype(np.float32)

    P = np.zeros((128, 128), dtype=np.float32)
    for o in (0, 64):
        P[o + np.arange(32), o + 32 + np.arange(32)] = 1.0
        P[o + 32 + np.arange(32), o + np.arange(32)] = -1.0
    p2t = np.ascontiguousarray(P.T)

    selms = np.zeros((128, 2), dtype=np.float32)
    selms[0:64, 0] = 1.0 / D
    selms[64:128, 1] = 1.0 / D
    selbc = np.zeros((2, 128), dtype=np.float32)
    selbc[0, 0:64] = 1.0
    selbc[1, 64:128] = 1.0
    ident = np.vstack([np.eye(64, dtype=np.float32)] * 2)      # [128, 64]

    in_maps = []
    for core in range(N_CORES):
        h0 = 2 * core
        wq_s = W_attn[:, 0 * C + h0 * D:0 * C + (h0 + 2) * D]
        wk_s = W_attn[:, 1 * C + h0 * D:1 * C + (h0 + 2) * D]
        wv_s = W_attn[:, 2 * C + h0 * D:2 * C + (h0 + 2) * D]
        bq_s = b_attn[0 * C + h0 * D:0 * C + (h0 + 2) * D]
        bk_s = b_attn[1 * C + h0 * D:1 * C + (h0 + 2) * D]
        bv_s = b_attn[2 * C + h0 * D:2 * C + (h0 + 2) * D]
        in_maps.append({
            "xT": xT,
            "wq": np.ascontiguousarray(wq_s),
            "wk": np.ascontiguousarray(wk_s),
            "wv": np.ascontiguousarray(wv_s),
            "bq": bq_s.reshape(128, 1).astype(np.float32),
            "bk": bk_s.reshape(128, 1).astype(np.float32),
            "bv": bv_s.reshape(128, 1).astype(np.float32),
            "wp": np.ascontiguousarray(W_proj[h0 * D:(h0 + 2) * D, :]),
            "cosb": cosb, "sinb": sinb,
            "p2t": p2t, "selms": selms, "selbc": selbc, "ident": ident,
            "ones1": np.ones((128, 1), dtype=np.float32),
        })
    return in_maps


_NC_CACHE = {}


def kernel(x, cos, sin, W_attn, b_attn, W_proj, b_proj):
    in_maps = host_inputs(np.asarray(x, dtype=np.float32), np.asarray(cos),
                          np.asarray(sin), np.asarray(W_attn),
                          np.asarray(b_attn), np.asarray(W_proj))
    if 'nc' not in _NC_CACHE:
        _NC_CACHE['nc'] = build_nc()
    nc = _NC_CACHE['nc']
    res = run_bass_kernel_spmd(nc, in_maps, core_ids=list(range(N_CORES)))
    acc = np.zeros((NTOK, C), dtype=np.float32)
    for r in res.results:
        acc += r["out"]
    return (acc + np.asarray(b_proj, dtype=np.float32)).reshape(B, T, C)
